# revision 1
# baseline (speedup 1.0000x reference)
"""Multi-head attention (B=2, S=2048, D=1024, H=16) on 8 Trainium2 cores.

Sharding: core c handles (batch b = c//4, head-group g = c%4 of 4 heads).
Megatron-style: W_q/k/v rows (output dims) column-sharded per head-group;
W_o columns row-sharded; the all-reduce over head-groups happens on the host
at gather time (sum of 4 partial projections per batch), where b_o is added.

Device layout (per core):
  qt/kt inputs are host-pre-transposed  X^T [1024, 2048]  (d_model, seq).
  Stage 1 computes q^T, k^T [256, 2048] (local dims on partitions, pair-
  chunked) and v [2048, 260] (seq on partitions; per head 64 cols + a ones
  column used to accumulate softmax row-sums during the A@V matmul).
  Attention per head: scores^T [j, i] via K=64 matmuls (heads auto row-tile
  via base partitions 0/64), exp on ScalarE straight out of PSUM (scale=1/8,
  no max subtraction: scores ~ N(0,1), fp32 exp is safe), A@V with the
  ones-augmented V so PSUM row 64 carries the row-sums.
  Normalization deferred: one batched ACT reciprocal at the end, gpsimd
  partition-broadcast, in-place DVE multiply.  Output projection with W_o^T
  chunked per head (K=64), partial result written as [1024, 2048] (e, s).

All matmuls run in float32r (TF32-like, full PE rate at N>=256).
"""

import numpy as np
from contextlib import ExitStack

import concourse.bass as bass
import concourse.bacc as bacc
import concourse.tile as tile
from concourse import mybir
from concourse.bass_utils import run_bass_kernel_spmd

F32 = mybir.dt.float32
F32R = mybir.dt.float32r
BF16 = mybir.dt.bfloat16
AF = mybir.ActivationFunctionType

B, S, D = 2, 2048, 1024
H, DH = 16, 64
NCORES = 8
LOC = D // 4          # 256 local dims per head-group
SCALE = 1.0 / np.sqrt(DH)

_CACHED_NC = None


def build_nc():
    nc = bacc.Bacc("TRN2", target_bir_lowering=False, debug=False)

    qt = nc.dram_tensor("qt", [D, S], F32R, kind="ExternalInput").ap()
    kt = nc.dram_tensor("kt", [D, S], F32R, kind="ExternalInput").ap()
    vt = nc.dram_tensor("vt", [D, S], F32R, kind="ExternalInput").ap()
    wqt = nc.dram_tensor("wqt", [D, LOC], F32R, kind="ExternalInput").ap()
    wkt = nc.dram_tensor("wkt", [D, LOC], F32R, kind="ExternalInput").ap()
    wvt = nc.dram_tensor("wvt", [D, LOC], F32R, kind="ExternalInput").ap()
    wot = nc.dram_tensor("wot", [DH, 4, D], BF16, kind="ExternalInput").ap()
    bq = nc.dram_tensor("bq", [128, 2], F32, kind="ExternalInput").ap()
    bk = nc.dram_tensor("bk", [128, 2], F32, kind="ExternalInput").ap()
    bv = nc.dram_tensor("bv", [128, LOC], F32, kind="ExternalInput").ap()
    vones = nc.dram_tensor("vones", [128, 16, 4], BF16, kind="ExternalInput").ap()
    outp = nc.dram_tensor("outp", [D, S], F32, kind="ExternalOutput").ap()

    with tile.TileContext(nc) as tc:
        with ExitStack() as ctx:
            wsb = ctx.enter_context(tc.tile_pool(name="wsb", bufs=1))
            big = ctx.enter_context(tc.tile_pool(name="big", bufs=1))

            # persistent SBUF state
            qt_sb = big.tile([128, 2, S], BF16, name="qt_sb")
            kt_sb = big.tile([128, 2, S], BF16, name="kt_sb")
            v_sb = big.tile([128, 16, 4, DH + 1], BF16, name="v_sb")
            ctx_sb = big.tile([64, 4, S], BF16, name="ctx_sb")
            # row 64: raw softmax row-sums (written from PSUM partition 64);
            # row 0: their reciprocals (written back by the unpack DMA)
            rs_sb = big.tile([65, 16, 512], F32, name="rs_sb")
            rs_pack = big.tile([128, 64], F32, name="rs_pack")
            rr_pack = big.tile([128, 64], F32, name="rr_pack")

            wq_sb = wsb.tile([128, 8, LOC], F32R, name="wq_sb")
            wk_sb = wsb.tile([128, 8, LOC], F32R, name="wk_sb")
            wv_sb = wsb.tile([128, 8, LOC], F32R, name="wv_sb")
            wo_sb = wsb.tile([DH, 4, D], BF16, name="wo_sb")
            bq_sb = wsb.tile([128, 2], F32, name="bq_sb")
            bk_sb = wsb.tile([128, 2], F32, name="bk_sb")
            bv_sb = wsb.tile([128, LOC], F32, name="bv_sb")

            nc.sync.dma_start(out=wq_sb, in_=wqt.rearrange("(a p) r -> p a r", p=128))
            nc.sync.dma_start(out=wk_sb, in_=wkt.rearrange("(a p) r -> p a r", p=128))
            nc.sync.dma_start(out=wv_sb, in_=wvt.rearrange("(a p) r -> p a r", p=128))
            nc.sync.dma_start(out=wo_sb, in_=wot)
            nc.sync.dma_start(out=bq_sb, in_=bq)
            nc.sync.dma_start(out=bk_sb, in_=bk)
            nc.sync.dma_start(out=bv_sb, in_=bv)

            # ones column of v (accumulates softmax row-sums in A@V)
            nc.sync.dma_start(out=v_sb[:, :, :, DH : DH + 1], in_=vones)

            # ---- Phase A: v projection (natural layout, s on partitions) ----
            bv3 = bv_sb.rearrange("p (h d) -> p h d", h=4)
            with (
                tc.tile_pool(name="vin", bufs=4) as vin,
                tc.tile_pool(name="vps", bufs=6, space="PSUM") as vps,
            ):
                for sg in range(4):  # groups of 4 s-chunks (512 rows of seq)
                    psv = [
                        vps.tile([128, LOC], F32, name="psv") for _ in range(4)
                    ]
                    for ds in range(8):
                        vt_t = vin.tile([128, 512], F32R, name="vt_t")
                        nc.scalar.dma_start(
                            out=vt_t,
                            in_=vt[ds * 128 : (ds + 1) * 128,
                                   sg * 512 : (sg + 1) * 512],
                        )
                        for c in range(4):
                            nc.tensor.matmul(
                                psv[c],
                                lhsT=vt_t[:, c * 128 : (c + 1) * 128],
                                rhs=wv_sb[:, ds, :],
                                start=(ds == 0),
                                stop=(ds == 7),
                            )
                    for c in range(4):
                        sc = sg * 4 + c
                        nc.vector.tensor_add(
                            v_sb[:, sc, :, 0:DH],
                            psv[c].rearrange("p (h d) -> p h d", h=4),
                            bv3,
                        )

            # ---- Phase B: q/k projections (transposed, local dims on parts) --
            with (
                tc.tile_pool(name="qkin", bufs=6) as qkin,
                tc.tile_pool(name="qkps", bufs=6, space="PSUM") as qkps,
            ):
                for st in range(4):  # s-tiles of 512
                    ps = {}
                    for t in range(2):
                        for pr in range(2):
                            ps[t, pr] = qkps.tile([128, 512], F32, name="psqk")
                    for ds in range(8):
                        qt_t = qkin.tile([128, 512], F32R, name="qt_t")
                        kt_t = qkin.tile([128, 512], F32R, name="kt_t")
                        nc.sync.dma_start(
                            out=qt_t,
                            in_=qt[ds * 128 : (ds + 1) * 128,
                                   st * 512 : (st + 1) * 512],
                        )
                        nc.sync.dma_start(
                            out=kt_t,
                            in_=kt[ds * 128 : (ds + 1) * 128,
                                   st * 512 : (st + 1) * 512],
                        )
                        for pr in range(2):
                            nc.tensor.matmul(
                                ps[0, pr],
                                lhsT=wq_sb[:, ds, pr * 128 : (pr + 1) * 128],
                                rhs=qt_t,
                                start=(ds == 0),
                                stop=(ds == 7),
                            )
                            nc.tensor.matmul(
                                ps[1, pr],
                                lhsT=wk_sb[:, ds, pr * 128 : (pr + 1) * 128],
                                rhs=kt_t,
                                start=(ds == 0),
                                stop=(ds == 7),
                            )
                    for pr in range(2):
                        nc.scalar.activation(
                            out=qt_sb[:, pr, st * 512 : (st + 1) * 512],
                            in_=ps[0, pr],
                            func=AF.Identity,
                            bias=bq_sb[:, pr : pr + 1],
                            scale=1.0,
                        )
                        nc.scalar.activation(
                            out=kt_sb[:, pr, st * 512 : (st + 1) * 512],
                            in_=ps[1, pr],
                            func=AF.Identity,
                            bias=bk_sb[:, pr : pr + 1],
                            scale=1.0,
                        )

            # ---- Phase C: attention (scores^T, exp, ones-augmented A@V) -----
            with (
                tc.tile_pool(name="expp", bufs=4) as expp,
                tc.tile_pool(name="qk2ps", bufs=2, space="PSUM") as qk2ps,
                tc.tile_pool(name="avps", bufs=4, space="PSUM") as avps,
            ):
                for pr in range(2):
                    for ih in range(2):  # i halves of 1024 query columns
                        psav = {
                            (hh, it): avps.tile([DH + 1, 512], F32, name="psav")
                            for hh in range(2)
                            for it in range(2)
                        }
                        # one-deep software pipeline: AV(jc) is emitted
                        # after QK(jc+1), so the PE always has QK work in
                        # flight while ACT computes exp and never stalls
                        def emit_qk(jc):
                            psqk = {}
                            for hh in range(2):
                                r0, r1 = hh * 64, (hh + 1) * 64
                                psqk[hh] = qk2ps.tile(
                                    [128, 1024], F32, name="psqk2"
                                )
                                for it in range(2):
                                    i0 = ih * 1024 + it * 512
                                    nc.tensor.matmul(
                                        psqk[hh][:, it * 512 : (it + 1) * 512],
                                        lhsT=kt_sb[r0:r1, pr,
                                                   jc * 128 : (jc + 1) * 128],
                                        rhs=qt_sb[r0:r1, pr, i0 : i0 + 512],
                                        start=True,
                                        stop=True,
                                    )
                            return psqk

                        def emit_exp_av(psqk, jc):
                            for hh in range(2):
                                ex = expp.tile([128, 1024], BF16, name="ex")
                                nc.scalar.activation(
                                    out=ex, in_=psqk[hh], func=AF.Exp,
                                    scale=SCALE,
                                )
                                for it in range(2):
                                    nc.tensor.matmul(
                                        psav[hh, it],
                                        lhsT=v_sb[:, jc, 2 * pr + hh, :],
                                        rhs=ex[:, it * 512 : (it + 1) * 512],
                                        start=(jc == 0),
                                        stop=(jc == 15),
                                    )

                        # warm-keeper: cheap HAM-visible matmuls bridge
                        # the PE over the i-half boundary stall so the clock
                        # gate stays at 8/8 (results are overwritten by the
                        # real start=True QK matmuls into the same slot)
                        warm = qk2ps.tile([128, 1024], F32, name="psqk2")
                        for w in range(24):
                            nc.tensor.matmul(
                                warm[:, 0:128],
                                lhsT=kt_sb[0:64, pr, 0:128],
                                rhs=qt_sb[0:64, pr, 0:128],
                                start=True,
                                stop=True,
                            )
                        prev = emit_qk(0)
                        for jc in range(1, 16):
                            cur = emit_qk(jc)
                            emit_exp_av(prev, jc - 1)
                            prev = cur
                        emit_exp_av(prev, 15)
                        # write unnormalized context + stash row-sums
                        for hh in range(2):
                            h = 2 * pr + hh
                            for it in range(2):
                                i0 = ih * 1024 + it * 512
                                slot = ((pr * 2 + ih) * 2 + hh) * 2 + it
                                nc.vector.tensor_copy(
                                    ctx_sb[:, h, i0 : i0 + 512],
                                    psav[hh, it][0:DH, :],
                                )
                                nc.vector.tensor_copy(
                                    rs_sb[64:65, slot, :],
                                    psav[hh, it][DH : DH + 1, :],
                                )

            # ---- batched reciprocal + deferred normalization ---------------
            # spread the 16x512 row-sums over 128 partitions so the DVE
            # iterative divide runs 128 lanes wide, then restore row layout
            nc.sync.dma_start(
                out=rs_pack,
                in_=rs_sb[64:65, :, :].rearrange("p a b -> p (a b)"),
            )
            nc.vector.reciprocal(rr_pack, rs_pack)
            nc.sync.dma_start(
                out=rs_sb[0:1, :, :].rearrange("p a b -> p (a b)"),
                in_=rr_pack,
            )
            with tc.tile_pool(name="normp", bufs=4) as normp:
                for pr in range(2):
                    for ih in range(2):
                        for hh in range(2):
                            h = 2 * pr + hh
                            for it in range(2):
                                i0 = ih * 1024 + it * 512
                                slot = ((pr * 2 + ih) * 2 + hh) * 2 + it
                                rb = normp.tile([64, 512], F32, name="rb")
                                nc.gpsimd.partition_broadcast(
                                    rb, rs_sb[0:1, slot, :]
                                )
                                nc.vector.tensor_mul(
                                    ctx_sb[:, h, i0 : i0 + 512],
                                    ctx_sb[:, h, i0 : i0 + 512],
                                    rb,
                                )

            # ---- Phase D: output projection (partial over local dims) ------
            with (
                tc.tile_pool(name="pob", bufs=4) as pob,
                tc.tile_pool(name="pps", bufs=4, space="PSUM") as pps,
            ):
                for ec in range(8):  # output-dim chunks of 128
                    for st in range(4):  # s-tiles of 512
                        pp = pps.tile([128, 512], F32, name="pp")
                        for hc in range(4):
                            nc.tensor.matmul(
                                pp,
                                lhsT=wo_sb[:, hc, ec * 128 : (ec + 1) * 128],
                                rhs=ctx_sb[:, hc, st * 512 : (st + 1) * 512],
                                start=(hc == 0),
                                stop=(hc == 3),
                            )
                        ob = pob.tile([128, 512], F32, name="ob")
                        nc.vector.tensor_copy(ob, pp)
                        nc.sync.dma_start(
                            out=outp[ec * 128 : (ec + 1) * 128,
                                     st * 512 : (st + 1) * 512],
                            in_=ob,
                        )

    nc.compile()
    return nc


def _get_nc():
    global _CACHED_NC
    if _CACHED_NC is None:
        _CACHED_NC = build_nc()
    return _CACHED_NC


def make_in_maps(Q, K, V, W_q, b_q, W_k, b_k, W_v, b_v, W_o):
    xt = {}
    for b in range(B):
        xt["q", b] = np.ascontiguousarray(np.asarray(Q[b], np.float32).T)
        xt["k", b] = np.ascontiguousarray(np.asarray(K[b], np.float32).T)
        xt["v", b] = np.ascontiguousarray(np.asarray(V[b], np.float32).T)
    in_maps = []
    for c in range(NCORES):
        b, g = divmod(c, 4)
        L = slice(g * LOC, (g + 1) * LOC)
        wqt = np.ascontiguousarray(np.asarray(W_q, np.float32)[L, :].T)
        wkt = np.ascontiguousarray(np.asarray(W_k, np.float32)[L, :].T)
        wvt = np.ascontiguousarray(np.asarray(W_v, np.float32)[L, :].T)
        import ml_dtypes
        wot = np.ascontiguousarray(
            np.asarray(W_o, np.float32)[:, L].T.reshape(4, DH, D)
            .transpose(1, 0, 2).astype(ml_dtypes.bfloat16)
        )
        bqh = np.ascontiguousarray(np.asarray(b_q, np.float32)[L].reshape(2, 128).T)
        bkh = np.ascontiguousarray(np.asarray(b_k, np.float32)[L].reshape(2, 128).T)
        bvh = np.ascontiguousarray(
            np.broadcast_to(np.asarray(b_v, np.float32)[L], (128, LOC))
        )
        in_maps.append(
            dict(
                qt=xt["q", b], kt=xt["k", b], vt=xt["v", b],
                wqt=wqt, wkt=wkt, wvt=wvt, wot=wot,
                bq=bqh, bk=bkh, bv=bvh,
                vones=np.ones((128, 16, 4), __import__('ml_dtypes').bfloat16),
            )
        )
    return in_maps


def gather(results, b_o):
    out = np.zeros((B, S, D), dtype=np.float32)
    for c in range(NCORES):
        b = c // 4
        out[b] += results[c]["outp"].T
    out += np.asarray(b_o, np.float32)
    return out


def kernel(Q, K, V, W_q, b_q, W_k, b_k, W_v, b_v, W_o, b_o):
    nc = _get_nc()
    in_maps = make_in_maps(Q, K, V, W_q, b_q, W_k, b_k, W_v, b_v, W_o)
    res = run_bass_kernel_spmd(nc, in_maps, core_ids=list(range(NCORES)))
    return gather(res.results, b_o)



# revision 4
# speedup vs baseline: 1.0015x; 1.0015x over previous
"""Multi-head attention (B=2, S=2048, D=1024, H=16) on 8 Trainium2 cores.

Sharding: core c handles (batch b = c//4, head-group g = c%4 of 4 heads).
Megatron-style: W_q/k/v rows (output dims) column-sharded per head-group;
W_o columns row-sharded; the all-reduce over head-groups happens on the host
at gather time (sum of 4 partial projections per batch), where b_o is added.

All device data is bf16 (fp32 PSUM accumulation), halving HBM traffic and
PE input bandwidth vs fp32r.  Host pre-transposes X^T and pre-casts.

Device layout (per core):
  Warmup dummy matmuls run while the first DMAs land so the PE HAM clock
  gate is at 8/8 (2.4 GHz) when real work starts.
  Phase A: v projection, natural layout (seq on partitions), v stored
  [128, 16 s-chunks, 4 heads, 65] with a ones column per head that makes
  the A@V matmul accumulate softmax row-sums in PSUM row 64.
  Phase B: q/k projections transposed (local dims on partitions, pair-
  chunked): q^T,k^T [256, 2048] as [128, 2, S].
  Phase C: per (pr, ih, hh) pass: scores^T [j, i] via K=64 matmuls (row
  tiling via base partitions 0/64), exp on ScalarE straight out of PSUM
  (scale=1/8, no max subtraction: scores ~ N(0,1)), ones-augmented A@V.
  ScalarE does nothing but exp (the phase-C bottleneck); normalization is
  per-pass and fully overlapped: DVE rowsum copy -> gpsimd partition
  broadcast -> DVE reciprocal_approx_fast -> fused DVE (PSUM * rinv) ->
  bf16 ctx write.
  Phase D: output projection from ctx [64, 4, S], partials [1024, 2048]
  fp32 DMA'd out per tile.
"""

import numpy as np
from contextlib import ExitStack

import concourse.bass as bass
import concourse.bacc as bacc
import concourse.tile as tile
from concourse import mybir
from concourse.bass_utils import run_bass_kernel_spmd

F32 = mybir.dt.float32
BF16 = mybir.dt.bfloat16
AF = mybir.ActivationFunctionType

B, S, D = 2, 2048, 1024
H, DH = 16, 64
NCORES = 8
LOC = D // 4          # 256 local dims per head-group
SCALE = 1.0 / np.sqrt(DH)

_CACHED_NC = None


def build_nc():
    nc = bacc.Bacc("TRN2", target_bir_lowering=False, debug=False)

    qt = nc.dram_tensor("qt", [D, S], BF16, kind="ExternalInput").ap()
    kt = nc.dram_tensor("kt", [D, S], BF16, kind="ExternalInput").ap()
    vt = nc.dram_tensor("vt", [D, S], BF16, kind="ExternalInput").ap()
    wqt = nc.dram_tensor("wqt", [D, LOC], BF16, kind="ExternalInput").ap()
    wkt = nc.dram_tensor("wkt", [D, LOC], BF16, kind="ExternalInput").ap()
    wvt = nc.dram_tensor("wvt", [D, LOC], BF16, kind="ExternalInput").ap()
    wot = nc.dram_tensor("wot", [DH, 4, D], BF16, kind="ExternalInput").ap()
    bq = nc.dram_tensor("bq", [128, 2], F32, kind="ExternalInput").ap()
    bk = nc.dram_tensor("bk", [128, 2], F32, kind="ExternalInput").ap()
    bv = nc.dram_tensor("bv", [128, LOC], F32, kind="ExternalInput").ap()
    outp = nc.dram_tensor("outp", [D, S], F32, kind="ExternalOutput").ap()

    with tile.TileContext(nc) as tc:
        with ExitStack() as ctx:
            wsb = ctx.enter_context(tc.tile_pool(name="wsb", bufs=1))
            big = ctx.enter_context(tc.tile_pool(name="big", bufs=1))

            # persistent SBUF state
            qt_sb = big.tile([128, 2, S], BF16, name="qt_sb")
            kt_sb = big.tile([128, 2, S], BF16, name="kt_sb")
            v_sb = big.tile([128, 16, 4, DH + 1], BF16, name="v_sb")
            ctx_sb = big.tile([64, 4, S], BF16, name="ctx_sb")

            wq_sb = wsb.tile([128, 8, LOC], BF16, name="wq_sb")
            wk_sb = wsb.tile([128, 8, LOC], BF16, name="wk_sb")
            wv_sb = wsb.tile([128, 8, LOC], BF16, name="wv_sb")
            wo_sb = wsb.tile([DH, 4, D], BF16, name="wo_sb")
            bq_sb = wsb.tile([128, 2], F32, name="bq_sb")
            bk_sb = wsb.tile([128, 2], F32, name="bk_sb")
            bv_sb = wsb.tile([128, LOC], F32, name="bv_sb")
            wup = wsb.tile([64, 128], BF16, name="wup")

            nc.gpsimd.memset(wup, 0.0)
            # ones column of v (accumulates softmax row-sums in A@V)
            nc.gpsimd.memset(v_sb[:, :, :, DH : DH + 1], 1.0)

            # weight DMAs split per 128-row chunk so consumers can start
            # as soon as their chunk lands; wv first (phase A needs it)
            for ds in range(8):
                nc.sync.dma_start(
                    out=wv_sb[:, ds, :], in_=wvt[ds * 128 : (ds + 1) * 128, :]
                )
            for ds in range(8):
                nc.sync.dma_start(
                    out=wq_sb[:, ds, :], in_=wqt[ds * 128 : (ds + 1) * 128, :]
                )
                nc.sync.dma_start(
                    out=wk_sb[:, ds, :], in_=wkt[ds * 128 : (ds + 1) * 128, :]
                )
            nc.sync.dma_start(out=wo_sb, in_=wot)
            nc.sync.dma_start(out=bq_sb, in_=bq)
            nc.sync.dma_start(out=bk_sb, in_=bk)
            nc.sync.dma_start(out=bv_sb, in_=bv)

            # ---- Warmup: dummy matmuls while input DMAs land, so the PE
            # HAM clock gate reaches 8/8 before phase A ----
            with tc.tile_pool(name="wps", bufs=1, space="PSUM") as wps:
                wp = wps.tile([64, 128], F32, name="wp")
                for _ in range(36):
                    nc.tensor.matmul(
                        wp, lhsT=wup[:, 0:64], rhs=wup, start=True, stop=True
                    )

            # ---- Phase A: v projection (natural layout, s on partitions) ----
            bv3 = bv_sb.rearrange("p (h d) -> p h d", h=4)
            with (
                tc.tile_pool(name="vin", bufs=4) as vin,
                tc.tile_pool(name="vps", bufs=4, space="PSUM") as vps,
            ):
                for sg in range(4):  # groups of 4 s-chunks (512 rows of seq)
                    psv = [
                        vps.tile([128, LOC], F32, name="psv") for _ in range(4)
                    ]
                    for ds in range(8):
                        vt_t = vin.tile([128, 512], BF16, name="vt_t")
                        nc.scalar.dma_start(
                            out=vt_t,
                            in_=vt[ds * 128 : (ds + 1) * 128,
                                   sg * 512 : (sg + 1) * 512],
                        )
                        for c in range(4):
                            nc.tensor.matmul(
                                psv[c],
                                lhsT=vt_t[:, c * 128 : (c + 1) * 128],
                                rhs=wv_sb[:, ds, :],
                                start=(ds == 0),
                                stop=(ds == 7),
                            )
                    for c in range(4):
                        sc = sg * 4 + c
                        nc.vector.tensor_add(
                            v_sb[:, sc, :, 0:DH],
                            psv[c].rearrange("p (h d) -> p h d", h=4),
                            bv3,
                        )

            # ---- Phase B: q/k projections (transposed, local dims on parts) --
            with (
                tc.tile_pool(name="qkin", bufs=6) as qkin,
                tc.tile_pool(name="qkps", bufs=4, space="PSUM") as qkps,
            ):
                for st in range(4):  # s-tiles of 512
                    ps = {}
                    for t in range(2):
                        for pr in range(2):
                            ps[t, pr] = qkps.tile([128, 512], F32, name="psqk")
                    for ds in range(8):
                        qt_t = qkin.tile([128, 512], BF16, name="qt_t")
                        kt_t = qkin.tile([128, 512], BF16, name="kt_t")
                        nc.sync.dma_start(
                            out=qt_t,
                            in_=qt[ds * 128 : (ds + 1) * 128,
                                   st * 512 : (st + 1) * 512],
                        )
                        nc.scalar.dma_start(
                            out=kt_t,
                            in_=kt[ds * 128 : (ds + 1) * 128,
                                   st * 512 : (st + 1) * 512],
                        )
                        for pr in range(2):
                            nc.tensor.matmul(
                                ps[0, pr],
                                lhsT=wq_sb[:, ds, pr * 128 : (pr + 1) * 128],
                                rhs=qt_t,
                                start=(ds == 0),
                                stop=(ds == 7),
                            )
                            nc.tensor.matmul(
                                ps[1, pr],
                                lhsT=wk_sb[:, ds, pr * 128 : (pr + 1) * 128],
                                rhs=kt_t,
                                start=(ds == 0),
                                stop=(ds == 7),
                            )
                    # PSUM -> SBUF bf16 with per-partition bias, off ScalarE
                    for pr in range(2):
                        nc.vector.tensor_scalar_add(
                            qt_sb[:, pr, st * 512 : (st + 1) * 512],
                            ps[0, pr],
                            bq_sb[:, pr : pr + 1],
                        )
                        nc.vector.tensor_scalar_add(
                            kt_sb[:, pr, st * 512 : (st + 1) * 512],
                            ps[1, pr],
                            bk_sb[:, pr : pr + 1],
                        )

            # ---- Phase C: attention (scores^T, exp, ones-augmented A@V) -----
            with (
                tc.tile_pool(name="expp", bufs=4) as expp,
                tc.tile_pool(name="qk2ps", bufs=2, space="PSUM") as qk2ps,
                tc.tile_pool(name="avps", bufs=4, space="PSUM") as avps,
                tc.tile_pool(name="nrm", bufs=4) as nrm,
            ):
                for pr, ih in [(0, 0), (1, 0), (0, 1), (1, 1)]:
                    for hh in range(2):
                        h = 2 * pr + hh
                        r0, r1 = hh * 64, (hh + 1) * 64
                        psav = [
                            avps.tile([DH + 1, 512], F32, name="psav")
                            for _ in range(2)
                        ]

                        # one-deep software pipeline: AV(jc) is emitted after
                        # QK(jc+1) so the PE always has QK work in flight
                        # while ScalarE computes exp
                        def emit_qk(jc):
                            psqk = qk2ps.tile([128, 1024], F32, name="psqk2")
                            for it in range(2):
                                i0 = ih * 1024 + it * 512
                                nc.tensor.matmul(
                                    psqk[:, it * 512 : (it + 1) * 512],
                                    lhsT=kt_sb[r0:r1, pr,
                                               jc * 128 : (jc + 1) * 128],
                                    rhs=qt_sb[r0:r1, pr, i0 : i0 + 512],
                                    start=True,
                                    stop=True,
                                )
                            return psqk

                        def emit_exp_av(psqk, jc):
                            ex = expp.tile([128, 1024], BF16, name="ex")
                            nc.scalar.activation(
                                out=ex, in_=psqk, func=AF.Exp, scale=SCALE
                            )
                            for it in range(2):
                                nc.tensor.matmul(
                                    psav[it],
                                    lhsT=v_sb[:, jc, h, :],
                                    rhs=ex[:, it * 512 : (it + 1) * 512],
                                    start=(jc == 0),
                                    stop=(jc == 15),
                                )

                        prev = emit_qk(0)
                        for jc in range(1, 16):
                            cur = emit_qk(jc)
                            emit_exp_av(prev, jc - 1)
                            prev = cur
                        emit_exp_av(prev, 15)

                        # per-pass normalization, fully off ScalarE and
                        # overlapped with the next pass's matmuls
                        for it in range(2):
                            i0 = ih * 1024 + it * 512
                            rsum = nrm.tile([1, 512], F32, name="rsum")
                            nc.vector.tensor_copy(
                                rsum, psav[it][DH : DH + 1, :]
                            )
                            rb = nrm.tile([64, 512], F32, name="rb")
                            nc.gpsimd.partition_broadcast(rb, rsum)
                            rinv = nrm.tile([64, 512], F32, name="rinv")
                            nc.vector.reciprocal_approx_fast(out=rinv, in_=rb)
                            nc.vector.tensor_mul(
                                ctx_sb[:, h, i0 : i0 + 512],
                                psav[it][0:DH, :],
                                rinv,
                            )

            # ---- Phase D: output projection (partial over local dims) ------
            with (
                tc.tile_pool(name="pob", bufs=4) as pob,
                tc.tile_pool(name="pps", bufs=4, space="PSUM") as pps,
            ):
                for st in range(4):  # s-tiles of 512
                    for ec in range(8):  # output-dim chunks of 128
                        pp = pps.tile([128, 512], F32, name="pp")
                        for hc in range(4):
                            nc.tensor.matmul(
                                pp,
                                lhsT=wo_sb[:, hc, ec * 128 : (ec + 1) * 128],
                                rhs=ctx_sb[:, hc, st * 512 : (st + 1) * 512],
                                start=(hc == 0),
                                stop=(hc == 3),
                            )
                        ob = pob.tile([128, 512], F32, name="ob")
                        if ec % 2 == 0:
                            nc.vector.tensor_copy(ob, pp)
                            nc.sync.dma_start(
                                out=outp[ec * 128 : (ec + 1) * 128,
                                         st * 512 : (st + 1) * 512],
                                in_=ob,
                            )
                        else:
                            nc.scalar.activation(out=ob, in_=pp, func=AF.Copy)
                            nc.scalar.dma_start(
                                out=outp[ec * 128 : (ec + 1) * 128,
                                         st * 512 : (st + 1) * 512],
                                in_=ob,
                            )

    nc.compile()
    return nc


def _get_nc():
    global _CACHED_NC
    if _CACHED_NC is None:
        _CACHED_NC = build_nc()
    return _CACHED_NC


def make_in_maps(Q, K, V, W_q, b_q, W_k, b_k, W_v, b_v, W_o):
    import ml_dtypes

    BF = ml_dtypes.bfloat16
    xt = {}
    for b in range(B):
        xt["q", b] = np.ascontiguousarray(np.asarray(Q[b], np.float32).T).astype(BF)
        xt["k", b] = np.ascontiguousarray(np.asarray(K[b], np.float32).T).astype(BF)
        xt["v", b] = np.ascontiguousarray(np.asarray(V[b], np.float32).T).astype(BF)
    in_maps = []
    for c in range(NCORES):
        b, g = divmod(c, 4)
        L = slice(g * LOC, (g + 1) * LOC)
        wqt = np.ascontiguousarray(np.asarray(W_q, np.float32)[L, :].T).astype(BF)
        wkt = np.ascontiguousarray(np.asarray(W_k, np.float32)[L, :].T).astype(BF)
        wvt = np.ascontiguousarray(np.asarray(W_v, np.float32)[L, :].T).astype(BF)
        wot = np.ascontiguousarray(
            np.asarray(W_o, np.float32)[:, L].T.reshape(4, DH, D)
            .transpose(1, 0, 2).astype(BF)
        )
        bqh = np.ascontiguousarray(np.asarray(b_q, np.float32)[L].reshape(2, 128).T)
        bkh = np.ascontiguousarray(np.asarray(b_k, np.float32)[L].reshape(2, 128).T)
        bvh = np.ascontiguousarray(
            np.broadcast_to(np.asarray(b_v, np.float32)[L], (128, LOC))
        )
        in_maps.append(
            dict(
                qt=xt["q", b], kt=xt["k", b], vt=xt["v", b],
                wqt=wqt, wkt=wkt, wvt=wvt, wot=wot,
                bq=bqh, bk=bkh, bv=bvh,
            )
        )
    return in_maps


def gather(results, b_o):
    out = np.zeros((B, S, D), dtype=np.float32)
    for c in range(NCORES):
        b = c // 4
        out[b] += results[c]["outp"].T
    out += np.asarray(b_o, np.float32)
    return out


def kernel(Q, K, V, W_q, b_q, W_k, b_k, W_v, b_v, W_o, b_o):
    nc = _get_nc()
    in_maps = make_in_maps(Q, K, V, W_q, b_q, W_k, b_k, W_v, b_v, W_o)
    res = run_bass_kernel_spmd(nc, in_maps, core_ids=list(range(NCORES)))
    return gather(res.results, b_o)
